# revision 32
# baseline (speedup 1.0000x reference)
"""Trainium2 Bass kernel for nn_BSHConv3D: spherical-harmonic 3^3 conv.

The whole module collapses to one dense 3D convolution
x[1,48,48,48,8] -> out[48,48,48, 512] with combined weights
W[3,3,3, 8, 512] (the central 1x1x1 conv folds into the center tap, the
bias rides on an extra constant-ones contraction row).

Per-core (D sharded 8 x 6 slabs, halo 1):
  - host builds the FULL 27-tap im2col: S[217, 14976] where row
    (kd,kh,kw,c) is the correspondingly shifted padded x volume and row
    216 is constant ones; z = flattened (d,h,w) padded coords
  - matmul per 128-position tile: 2 PSUM-accumulating matmuls
    (K = 128 + 89 contraction rows) x N=512 output channels
  - PE streams at 1 col / 1.2GHz-cycle here regardless of dtype, so
    2 matmuls/tile (1024 streamed cols) is the floor
  - PSUM evacuated by VectorE/ScalarE alternating into a 9-tile group
    staging buffer, one ~2.3MB output DMA per group (2KB descriptors
    spread across all 16 SDMA engines; bigger merged descriptors pin to
    one engine at ~26GB/s)
  - HBM->SBUF loads ride SWDGE (gpsimd) for the same reason
"""

from contextlib import ExitStack

import ml_dtypes
import numpy as np

import concourse.bass as bass
from concourse import bacc
import concourse.mybir as mybir
import concourse.tile as tile
from concourse.bass_utils import run_bass_kernel_spmd

B, D, H, W, C = 1, 48, 48, 48, 8
KS, R, DEG, NH, OUT = 3, 2, 3, 16, 16
NCORES = 8
DL = D // NCORES  # 6 output slabs per core
HP = WP = 50  # zero-padded H/W
SLAB = HP * WP  # 2500
NSLAB = DL + 2  # local slabs incl. halos
MARGIN = 64  # left margin in the host z buffer (shift slack)
UD = NSLAB * SLAB  # 20000 payload columns
SZ = 20352  # host-side padded z columns
NCH = OUT * NH * 2  # 512 output channels (f, n, re/im)
KC = 27 * C + 1  # 217 contraction rows: 27 taps x 8 ch + ones row
KA = 128  # contraction chunk A (SBUF partition limit)
KB = KC - KA  # 89
ZB0 = MARGIN + SLAB  # first computed z column (host coords)
TM = 128  # positions per matmul tile
NT = 117  # z tiles per core
NZ = NT * TM  # 14976 z columns materialized on chip
NVALID = DL * H * W  # 13824 valid output rows per core
NPAD = NZ  # padded output rows written
NZ_CHUNKS = 6  # im2col load chunking so matmuls can start early
GT = 9  # z tiles grouped per output DMA (117 = 13 groups of 9)

IO_DTYPE = "fp16"  # "fp16" | "bf16" | "f32r" matmul input dtype
OUT_DTYPE = "fp16"  # "fp16" | "f32" output DMA dtype (host upcasts)

# module-level knobs for the test harness (graders just call kernel())
TRACE = False
LAST_RESULTS = None


def _valid_row_index():
    """Indices into the padded [NPAD] output rows that are real outputs,
    in output raster order."""
    u = np.arange(NPAD) + (ZB0 - MARGIN)
    dl = u // SLAB
    hp = (u % SLAB) // WP
    wp = u % WP
    mask = (dl >= 1) & (dl < 1 + DL) & (hp >= 1) & (hp <= H) & (wp >= 1) & (wp <= W)
    idx = np.nonzero(mask)[0]
    assert idx.size == NVALID, idx.size
    return idx


_VALID_IDX = _valid_row_index()


_MDT = {"fp16": mybir.dt.float16, "bf16": mybir.dt.bfloat16, "f32r": mybir.dt.float32r}


def _build_program():
    f32 = mybir.dt.float32
    mdt = _MDT[IO_DTYPE]
    odt = mybir.dt.float16 if OUT_DTYPE == "fp16" else f32
    nc = bacc.Bacc("TRN2", debug=False)
    xin = nc.dram_tensor("xin", [KC, NZ], mdt, kind="ExternalInput").ap()
    wc = nc.dram_tensor("wc", [KC, NCH], mdt, kind="ExternalInput").ap()
    # output rows permuted [group][p][g][c] so each (partition, group) pair
    # is one contiguous GT*NCH-byte DMA descriptor; host unpermutes
    out = nc.dram_tensor(
        "out", [NT // GT, TM, GT, NCH], odt, kind="ExternalOutput"
    ).ap()

    with tile.TileContext(nc) as tc, ExitStack() as ctx:
        const_pool = ctx.enter_context(tc.tile_pool(name="const", bufs=1))
        stage_pool = ctx.enter_context(tc.tile_pool(name="stage", bufs=3))
        psum_pool = ctx.enter_context(tc.tile_pool(name="psum", bufs=8, space="PSUM"))

        SA = const_pool.tile([KA, NZ], mdt, name="SA")
        SB = const_pool.tile([KB, NZ], mdt, name="SB")
        WtA = const_pool.tile([KA, NCH], mdt, name="WtA")
        WtB = const_pool.tile([KB, NCH], mdt, name="WtB")

        # big loads ride SWDGE (gpsimd): the HWDGE path pins a load to a
        # single SDMA engine; SWDGE spreads across all 16. The small weight
        # loads go on idle sync HWDGE to keep the gpsimd issue queue clear.
        nc.sync.dma_start(WtA[:, :], wc[0:KA])
        nc.sync.dma_start(WtB[:, :], wc[KA:KC])

        # non-uniform chunks: small early chunks start the matmuls fast,
        # big later chunks amortize per-descriptor overhead
        lo = 0
        for frac in (16, 16, 8, 8, 8, 4, 4):
            hi = min(NZ, lo + NZ // frac)
            for p0, p1 in ((0, 32), (32, 64), (64, 96), (96, 128)):
                nc.gpsimd.dma_start(SA[p0:p1, lo:hi], xin[p0:p1, lo:hi])
            for p0, p1 in ((0, 45), (45, 89)):
                nc.gpsimd.dma_start(
                    SB[p0:p1, lo:hi], xin[KA + p0 : KA + p1, lo:hi]
                )
            lo = hi
        assert lo == NZ, lo
        del f32  # staging dtype below follows the output dtype

        for g0 in range(0, NT, GT):
            st = stage_pool.tile([TM, GT * NCH], odt, name="st")
            for g in range(GT):
                t = g0 + g
                zb = t * TM
                ps = psum_pool.tile([TM, NCH], mybir.dt.float32, name="ps")
                nc.tensor.matmul(
                    ps[:, :], SA[:, zb : zb + TM], WtA[:, :],
                    start=True, stop=False,
                )
                nc.tensor.matmul(
                    ps[:, :], SB[:, zb : zb + TM], WtB[:, :],
                    start=False, stop=True,
                )
                dst = st[:, g * NCH : (g + 1) * NCH]
                if t % 2 == 0:
                    nc.vector.tensor_copy(dst, ps[:, :])
                else:
                    nc.scalar.copy(dst, ps[:, :])
            # one DMA per group, both sides contiguous per partition; the
            # last group drains in 3-tile sub-DMAs to shorten the tail
            if g0 + GT < NT:
                nc.sync.dma_start(out[g0 // GT], st[:, :])
            else:
                for s in range(0, GT - 3, 3):
                    nc.sync.dma_start(
                        out[g0 // GT][:, s : s + 3, :],
                        st[:, s * NCH : (s + 3) * NCH],
                    )
                for s in range(GT - 3, GT):
                    nc.sync.dma_start(
                        out[g0 // GT][:, s : s + 1, :],
                        st[:, s * NCH : (s + 1) * NCH],
                    )
    nc.compile()
    return nc


_program_cache = {}


def _get_program():
    if "nc" not in _program_cache:
        _program_cache["nc"] = _build_program()
    return _program_cache["nc"]


def _host_weights(atoms_real, atoms_imag, w, w_center, b_center):
    idx = np.repeat(np.arange(DEG + 1), [2 * n + 1 for n in range(DEG + 1)])
    w_exp = w[..., idx]  # [C,F,R,NH]
    WR = np.einsum("dhwrn,cfrn->dhwcfn", atoms_real, w_exp)
    WI = np.einsum("dhwrn,cfrn->dhwcfn", atoms_imag, w_exp)
    Wfull = np.stack([WR, WI], axis=-1)  # [3,3,3,C,F,NH,2]
    Wc = np.zeros((KC, NCH), np.float32)
    Wc[: KC - 1, :] = Wfull.reshape(KC - 1, NCH)
    # central 1x1x1 conv onto (f, n=0, re): tap (kd=1,kh=1,kw=1) rows 104..111
    Wc[104:112, 0::32] += w_center
    Wc[KC - 1, 0::32] = b_center
    return Wc


def kernel(x, atoms_real, atoms_imag, w, w_center, b_center):
    global LAST_RESULTS
    x = np.asarray(x, np.float32)
    Wc = _host_weights(
        np.asarray(atoms_real, np.float32),
        np.asarray(atoms_imag, np.float32),
        np.asarray(w, np.float32),
        np.asarray(w_center, np.float32),
        np.asarray(b_center, np.float32),
    )
    hdt = {"fp16": np.float16, "bf16": ml_dtypes.bfloat16, "f32r": np.float32}[IO_DTYPE]
    Wc = Wc.astype(hdt)

    xt = np.transpose(x[0], (3, 0, 1, 2))  # [C,D,H,W]
    xpad = np.zeros((C, D + 2, HP, WP), np.float32)
    xpad[:, 1 : D + 1, 1 : H + 1, 1 : W + 1] = xt

    in_maps = []
    for core in range(NCORES):
        d0 = core * DL
        pbuf = np.zeros((C, SZ), np.float32)
        pbuf[:, MARGIN : MARGIN + UD] = xpad[:, d0 : d0 + NSLAB].reshape(C, UD)
        buf = np.empty((KC, NZ), np.float32)
        r = 0
        for kd in range(3):
            for kh in range(3):
                for kw in range(3):
                    off = (kd - 1) * SLAB + (kh - 1) * WP + (kw - 1)
                    buf[r : r + 8] = pbuf[:, ZB0 + off : ZB0 + off + NZ]
                    r += 8
        buf[KC - 1] = 1.0
        in_maps.append({"xin": buf.astype(hdt), "wc": Wc})

    nc = _get_program()
    res = run_bass_kernel_spmd(
        nc, in_maps, core_ids=list(range(NCORES)), trace=TRACE
    )
    LAST_RESULTS = res
    outs = [
        res.results[i]["out"]
        .transpose(0, 2, 1, 3)
        .reshape(NPAD, NCH)[_VALID_IDX]
        .astype(np.float32)
        for i in range(NCORES)
    ]
    full = np.concatenate([o.reshape(DL, H, W, OUT, NH, 2) for o in outs], axis=0)
    return full[None]
